# revision 1
# baseline (speedup 1.0000x reference)
"""Trainium2 Bass kernel for nn_DMHA_3255585210402 (retrieval_knn DMHA).

Key algebraic fact: TOPK == NVK == 4, so jax.lax.top_k over the size-4 v_keys
axis selects *all* entries; the gather+sum over (DVH, TOPK) therefore reduces
to a constant vector c = 2 * v_embed[0:4].sum(0), and the whole
compute_value_states branch collapses to  v = x * c  (verified: 1.4e-7 rel).

So the module is a causal MHA layer (B=2, H=16, T=2048, HD=128, D=2048) with
elementwise-scaled V.  Sharding: 8 cores = 2 batches x 4 head-groups.  Each
core computes, for its batch b and 4 heads:
  qT/kT projections (transposed layout, feature-on-partition),
  causal softmax attention in transposed score layout (sT[tk, tq]),
  the partial output projection  outT_g = Wo[:, gsl]-slice.T @ oT.
Host sums the 4 head-group partials per batch and adds bo.

The c scale rides the per-partition scalar of the normalize multiply
(o = c[p] * (x_g.T @ w) * recip[tq]), so V is never materialized.
All matmuls run as float32r; softmax denominators use the ones-column
matmul for the partition reduction and reciprocal_approx_fast + a
DMA row-broadcast so the PE never waits on the normalization chain.
"""

import math

import numpy as np

import concourse.bass as bass
import concourse.mybir as mybir
import concourse.tile as tile
from concourse import bacc
from concourse.bass_utils import run_bass_kernel_spmd

B, T, D = 2, 2048, 2048
H, HD = 16, 128
G = 4              # head-groups (cores per batch)
GH = H // G        # heads per core
GF = GH * HD       # projected features per core (512)
NCORES = 8
P = 128            # partitions
TQ = 512           # tq chunk width (psum bank / fp32 moving max)
F32 = mybir.dt.float32
F32R = mybir.dt.float32r

DK = D // P        # 16 contraction chunks for projections
NTQ = T // TQ      # 4 query chunks
NTK = T // P       # 16 key chunks


def _body(tc, xT, xg, wqT, wkT, woT, cT, bqT, bkT, ones, out):
    nc = tc.nc
    rsqrt_hd = 1.0 / math.sqrt(HD)
    mult = mybir.AluOpType.mult

    with (
        tc.tile_pool(name="const", bufs=1) as const,
        tc.tile_pool(name="res1", bufs=1) as res1,
    ):
        # preload the gpsimd library that partition_broadcast needs so the
        # ~11us library DMA happens during phase A, not at first use
        from concourse import library_config
        with tc.high_priority():
            nc.gpsimd.load_library(library_config.attn)
        qT_sb = res1.tile([P, GH, T], F32R)   # q, transposed per head
        kT_sb = res1.tile([P, GH, T], F32R)

        # --- phase A: q/k projections, transposed layout ---
        with (
            tc.tile_pool(name="wqk", bufs=1) as wqk,
            tc.tile_pool(name="xt", bufs=20) as xtp,
            tc.tile_pool(name="psA", bufs=8, space="PSUM") as psA,
        ):
            wq_sb = wqk.tile([P, DK, GF], F32R)
            wk_sb = wqk.tile([P, DK, GF], F32R)
            wqr = wqT.rearrange("(n p) f -> p n f", p=P)
            wkr = wkT.rearrange("(n p) f -> p n f", p=P)
            xts0 = []
            for dk in range(DK):
                nc.sync.dma_start(out=wq_sb[:, dk, :], in_=wqr[:, dk, :])
                nc.sync.dma_start(out=wk_sb[:, dk, :], in_=wkr[:, dk, :])
                xt0 = xtp.tile([P, TQ], F32R, name="xt")
                nc.sync.dma_start(
                    out=xt0, in_=xT[dk * P : (dk + 1) * P, 0:TQ]
                )
                xts0.append(xt0)

            # small constants (needed from the first psum copy onward)
            ones_sb = const.tile([P, P], F32R)
            nc.sync.dma_start(out=ones_sb, in_=ones)
            bq_sb = const.tile([HD, GH], F32)
            nc.sync.dma_start(out=bq_sb, in_=bqT)
            bk_sb = const.tile([HD, GH], F32)
            nc.sync.dma_start(out=bk_sb, in_=bkT)
            cT_sb = const.tile([HD, GH], F32)
            nc.sync.dma_start(out=cT_sb, in_=cT)

            for tci in range(NTQ):
                tsl = slice(tci * TQ, (tci + 1) * TQ)
                ps = [
                    psA.tile([P, TQ], F32, name="psA_t", tag="psA_t")
                    for _ in range(2 * GH)
                ]
                for dk in range(DK):
                    if tci == 0:
                        xt = xts0[dk]
                    else:
                        xt = xtp.tile([P, TQ], F32R, name="xt")
                        nc.sync.dma_start(
                            out=xt, in_=xT[dk * P : (dk + 1) * P, tsl]
                        )
                    for w, w_sb in enumerate((wq_sb, wk_sb)):
                        for h in range(GH):
                            nc.tensor.matmul(
                                ps[w * GH + h],
                                w_sb[:, dk, h * HD : (h + 1) * HD],
                                xt,
                                start=(dk == 0),
                                stop=(dk == DK - 1),
                            )
                for w, dstT, bias in ((0, qT_sb, bq_sb), (1, kT_sb, bk_sb)):
                    for h in range(GH):
                        nc.scalar.activation(
                            dstT[:, h, tsl],
                            ps[w * GH + h],
                            mybir.ActivationFunctionType.Identity,
                            bias=bias[:, h : h + 1],
                        )

        # --- phases B+C interleaved over query chunks ---
        with (
            tc.tile_pool(name="res2", bufs=1) as res2,
            tc.tile_pool(name="wt", bufs=6) as wtp,
            tc.tile_pool(name="pr", bufs=3) as prp,
            tc.tile_pool(name="small", bufs=4) as smp,
            tc.tile_pool(name="ct", bufs=4) as ctp,
            tc.tile_pool(name="psS", bufs=4, space="PSUM") as psS,
            tc.tile_pool(name="psO", bufs=2, space="PSUM") as psO,
            tc.tile_pool(name="psSum", bufs=2, space="PSUM") as psSum,
        ):
            xg_sb = res2.tile([P, NTK, GF], F32R)  # x[:, gsl] chunked by tk
            for i in range(NTK):
                nc.sync.dma_start(
                    out=xg_sb[:, i, :], in_=xg[i * P : (i + 1) * P, :]
                )
            oT_sb = res2.tile([P, GH, T], F32R)   # attention out, transposed
            wo_sb = res2.tile([P, GH, D], F32R)   # Wo[:, gsl].T chunked
            wor = woT.rearrange("(m p) d -> p m d", p=P)
            for m in range(GH):
                nc.sync.dma_start(out=wo_sb[:, m, :], in_=wor[:, m, :])

            pending = None
            for j in range(NTQ):
                qsl = slice(j * TQ, (j + 1) * TQ)
                nkk = (j + 1) * (TQ // P)  # causal: tk chunks needed
                # B: attention for each head on this query chunk
                for h in range(GH):
                    ps_o = psO.tile([P, TQ], F32, name="ps_o")
                    ps_sum = psSum.tile([1, TQ], F32, name="ps_sum")
                    wt_prev = None
                    for i in range(nkk):
                        ps_s = psS.tile([P, TQ], F32, name="ps_s", tag="ps_s")
                        nc.tensor.matmul(
                            ps_s,
                            kT_sb[:, h, i * P : (i + 1) * P],
                            qT_sb[:, h, qsl],
                            start=True,
                            stop=True,
                        )
                        wt = wtp.tile([P, TQ], F32R, name="wt")
                        nc.scalar.activation(
                            wt, ps_s, mybir.ActivationFunctionType.Exp,
                            scale=rsqrt_hd,
                        )
                        g = i - (TQ // P) * j
                        if g >= 0:  # diagonal tile: zero where tk > tq
                            nc.gpsimd.affine_select(
                                out=wt,
                                in_=wt,
                                pattern=[[1, TQ]],
                                compare_op=mybir.AluOpType.is_ge,
                                fill=0.0,
                                base=-(P * g),
                                channel_multiplier=-1,
                            )
                        nc.tensor.matmul(
                            ps_o,
                            xg_sb[:, i, h * HD : (h + 1) * HD],
                            wt,
                            start=(i == 0), stop=(i == nkk - 1),
                        )
                        # colsum: DVE pair-sums halve the PE's ones-matmuls
                        if i % 2 == 1:
                            wpair = prp.tile([P, TQ], F32R, name="wpair")
                            nc.vector.tensor_add(wpair, wt_prev, wt)
                            nc.tensor.matmul(
                                ps_sum, ones_sb[:, 0:1], wpair,
                                start=(i == 1), stop=(i == nkk - 1),
                            )
                        wt_prev = wt
                    # normalization (1/colsum -> partition broadcast ->
                    # (ps_o*c)*recip) is deferred one head so neither the
                    # gpsimd queue nor the PE ever waits on the chain
                    if pending is not None:
                        _emit_normalize(nc, smp, wtp, oT_sb, cT_sb, mult,
                                        *pending)
                    pending = (h, j, ps_o, ps_sum)
                # C: output projection, deferred one chunk so the PE
                # has B(j) queued while C(j-1)'s oT dependencies settle
                if j > 0:
                    _emit_outproj(nc, psS, ctp, wo_sb, oT_sb, out, j - 1)
            _emit_normalize(nc, smp, wtp, oT_sb, cT_sb, mult, *pending)
            _emit_outproj(nc, psS, ctp, wo_sb, oT_sb, out, NTQ - 1)


def _emit_normalize(nc, smp, wtp, oT_sb, cT_sb, mult, h, j, ps_o, ps_sum):
    """1/colsum on one partition, gpsimd partition broadcast, then
    (ps_o * c[p]) * recip in one DVE pass."""
    qsl = slice(j * TQ, (j + 1) * TQ)
    recip = smp.tile([1, TQ], F32, name="recip")
    nc.vector.reciprocal_approx_fast(out=recip, in_=ps_sum)
    rb = wtp.tile([P, TQ], F32, name="rb")
    nc.gpsimd.partition_broadcast(rb, recip)
    nc.vector.scalar_tensor_tensor(
        out=oT_sb[:, h, qsl],
        in0=ps_o,
        scalar=cT_sb[:, h : h + 1],
        in1=rb,
        op0=mult,
        op1=mult,
    )


def _emit_outproj(nc, psS, ctp, wo_sb, oT_sb, out, j):
    qsl = slice(j * TQ, (j + 1) * TQ)
    for dk in range(DK):
        ps = psS.tile([P, TQ], F32, name="psC_t", tag="ps_s")
        for m in range(GH):
            nc.tensor.matmul(
                ps,
                wo_sb[:, m, dk * P : (dk + 1) * P],
                oT_sb[:, m, qsl],
                start=(m == 0),
                stop=(m == GH - 1),
            )
        ct = ctp.tile([P, TQ], F32, name="ct")
        nc.scalar.copy(ct, ps)
        nc.sync.dma_start(out=out[dk * P : (dk + 1) * P, qsl], in_=ct)


def build_program():
    nc = bacc.Bacc(
        "TRN2", target_bir_lowering=False, debug=False, num_devices=NCORES
    )
    f = F32
    xT = nc.dram_tensor("xT", [D, T], F32R, kind="ExternalInput").ap()
    xg = nc.dram_tensor("xg", [T, GF], F32R, kind="ExternalInput").ap()
    wqT = nc.dram_tensor("wqT", [D, GF], F32R, kind="ExternalInput").ap()
    wkT = nc.dram_tensor("wkT", [D, GF], F32R, kind="ExternalInput").ap()
    woT = nc.dram_tensor("woT", [GF, D], F32R, kind="ExternalInput").ap()
    cT = nc.dram_tensor("cT", [HD, GH], f, kind="ExternalInput").ap()
    bqT = nc.dram_tensor("bqT", [HD, GH], f, kind="ExternalInput").ap()
    bkT = nc.dram_tensor("bkT", [HD, GH], f, kind="ExternalInput").ap()
    ones = nc.dram_tensor("ones", [P, P], F32R, kind="ExternalInput").ap()
    out = nc.dram_tensor("out", [D, T], f, kind="ExternalOutput").ap()

    with tile.TileContext(nc) as tc:
        _body(tc, xT, xg, wqT, wkT, woT, cT, bqT, bkT, ones, out)
    nc.compile()
    return nc


def _causal_masks() -> np.ndarray:
    """mask[g][p, f] = 1 iff tk <= tq for boundary tile offset g*128."""
    p = np.arange(P)[:, None]
    f = np.arange(TQ)[None, :]
    return np.stack(
        [(f >= p + g * P).astype(np.float32) for g in range(G)], axis=0
    )


_NC_CACHE = None
LAST_RESULT = None
TRACE = False


def kernel(x, Wq, bq, Wk, bk, Wvq, bvq, v_keys, v_embed, Wo, bo):
    global _NC_CACHE, LAST_RESULT
    x = np.asarray(x, np.float32)
    Wq = np.asarray(Wq, np.float32)
    bq = np.asarray(bq, np.float32)
    Wk = np.asarray(Wk, np.float32)
    bk = np.asarray(bk, np.float32)
    v_embed = np.asarray(v_embed, np.float32)
    Wo = np.asarray(Wo, np.float32)
    bo = np.asarray(bo, np.float32)

    c = 2.0 * v_embed[:G].sum(axis=0)
    in_maps = []
    for core in range(NCORES):
        b, g = divmod(core, G)
        gsl = slice(g * GF, (g + 1) * GF)
        in_maps.append(
            {
                "xT": np.ascontiguousarray(x[b].T),
                "xg": np.ascontiguousarray(x[b][:, gsl]),
                "wqT": np.ascontiguousarray(Wq[gsl, :].T),
                "wkT": np.ascontiguousarray(Wk[gsl, :].T),
                "woT": np.ascontiguousarray(Wo[:, gsl].T),
                "cT": np.ascontiguousarray(c[gsl].reshape(GH, HD).T),
                "bqT": np.ascontiguousarray(bq[gsl].reshape(GH, HD).T),
                "bkT": np.ascontiguousarray(bk[gsl].reshape(GH, HD).T),
                "ones": np.ones((P, P), np.float32),
            }
        )

    if _NC_CACHE is None:
        _NC_CACHE = build_program()
    res = run_bass_kernel_spmd(
        _NC_CACHE, in_maps, list(range(NCORES)), trace=TRACE
    )
    LAST_RESULT = res

    out = np.zeros((B, T, D), np.float32)
    for core in range(NCORES):
        b = core // G
        out[b] += res.results[core]["out"].T
    out += bo[None, None, :]
    return out


if __name__ == "__main__":
    nc = build_program()
    print("built ok")



# revision 3
# speedup vs baseline: 1.0466x; 1.0466x over previous
"""Trainium2 Bass kernel for nn_DMHA_3255585210402 (retrieval_knn DMHA).

Key algebraic fact: TOPK == NVK == 4, so jax.lax.top_k over the size-4 v_keys
axis selects *all* entries; the gather+sum over (DVH, TOPK) therefore reduces
to a constant vector c = 2 * v_embed[0:4].sum(0), and the whole
compute_value_states branch collapses to  v = x * c.

So the module is a causal MHA layer (B=2, H=16, T=2048, HD=128, D=2048) with
elementwise-scaled V.  Sharding: 8 cores = 2 batches x 4 head-groups.

All matmuls run in bf16 (same 1 cycle/row as fp32r on the PE but FWL weight
loads kick in, and ACT/DVE/SBUF byte traffic halves).  fp8 was measured and
rejected: softmax-logit noise from fp8 q/k lands directly on the output
(rel err 4.5e-2 > 2e-2 gate) because V is random -- no averaging rescue.

vs the fp32r baseline:
  * c is folded into the V operand on the host (xgc = x * c), so the o-psum
    drain is a plain tensor_tensor multiply with the recip broadcast.
  * exps are batched [128, 2, 512] across psum-bank pairs; column sums use
    DVE pair+quad adds so the PE only sees one ones-matmul per 4 chunks.
  * scores/o matmuls and the causal mask are trimmed to the live columns of
    diagonal tiles.
  * phase A streams x through SBUF dk-inner (one psum group per (w, head)),
    so drains pipeline on ACT and the PE never waits at chunk boundaries;
    initial weight DMAs are chunked so the first matmul starts early.
  * out-proj (phase C) is interleaved behind the attention head loop one
    query chunk back, and its psum drains run on the DVE.
"""

import math

import numpy as np
import ml_dtypes

import concourse.bass as bass
import concourse.mybir as mybir
import concourse.tile as tile
from concourse import bacc
from concourse.bass_utils import run_bass_kernel_spmd

B, T, D = 2, 2048, 2048
H, HD = 16, 128
G = 4              # head-groups (cores per batch)
GH = H // G        # heads per core
GF = GH * HD       # projected features per core (512)
NCORES = 8
P = 128            # partitions
TQ = 512           # tq chunk width (psum bank / fp32 moving max)
F32 = mybir.dt.float32
BF16 = mybir.dt.bfloat16

DK = D // P        # 16 contraction chunks for projections
NTQ = T // TQ      # 4 query chunks
NTK = T // P       # 16 key chunks


def _body(tc, xT, xgc, wqT, wkT, woT, bqT, bkT, ones, out):
    nc = tc.nc
    sc_exp = 1.0 / math.sqrt(HD)

    with (
        tc.tile_pool(name="const", bufs=1) as const,
        tc.tile_pool(name="res1", bufs=1) as res1,
    ):
        # preload the gpsimd library (affine_select / partition_broadcast)
        from concourse import library_config
        with tc.high_priority():
            nc.gpsimd.load_library(library_config.attn)

        qT_sb = res1.tile([P, GH, T], BF16)   # q, transposed per head
        kT_sb = res1.tile([P, GH, T], BF16)

        ones_sb = const.tile([P, P], BF16)
        nc.sync.dma_start(out=ones_sb, in_=ones)
        bq_sb = const.tile([HD, GH], F32)
        nc.sync.dma_start(out=bq_sb, in_=bqT)
        bk_sb = const.tile([HD, GH], F32)
        nc.sync.dma_start(out=bk_sb, in_=bkT)

        # --- phase A: q/k projections, bf16, dk-inner ---
        with (
            tc.tile_pool(name="wqk", bufs=1) as wqk,
            tc.tile_pool(name="xt", bufs=2) as xtp,
            tc.tile_pool(name="psA", bufs=4, space="PSUM") as psA,
        ):
            wq_sb = wqk.tile([P, DK, GF], BF16)
            wk_sb = wqk.tile([P, DK, GF], BF16)
            # chunk the first DMAs so the first matmul (dk=0) starts as soon
            # as the first slices land, not after 2 MB of weights
            xt0 = xtp.tile([P, DK, TQ], BF16, name="xt")
            for ch in range(4):
                dsl = slice(ch * 4, (ch + 1) * 4)
                nc.sync.dma_start(out=wq_sb[:, dsl, :], in_=wqT[:, dsl, :])
                nc.sync.dma_start(out=wk_sb[:, dsl, :], in_=wkT[:, dsl, :])
                nc.sync.dma_start(out=xt0[:, dsl, :], in_=xT[0][:, dsl, :])

            for tci in range(NTQ):
                tsl = slice(tci * TQ, (tci + 1) * TQ)
                if tci == 0:
                    xt = xt0
                else:
                    xt = xtp.tile([P, DK, TQ], BF16, name="xt")
                    nc.sync.dma_start(out=xt, in_=xT[tci])
                for w_sb, dstT, bias in (
                    (wq_sb, qT_sb, bq_sb),
                    (wk_sb, kT_sb, bk_sb),
                ):
                    for h in range(GH):
                        ps = psA.tile([P, TQ], F32, name="psA_t")
                        for dk in range(DK):
                            nc.tensor.matmul(
                                ps,
                                w_sb[:, dk, h * HD : (h + 1) * HD],
                                xt[:, dk, :],
                                start=(dk == 0),
                                stop=(dk == DK - 1),
                            )
                        nc.scalar.activation(
                            dstT[:, h, tsl],
                            ps,
                            mybir.ActivationFunctionType.Identity,
                            bias=bias[:, h : h + 1],
                        )

        # --- phases B+C interleaved over query chunks ---
        with (
            tc.tile_pool(name="res2", bufs=1) as res2,
            tc.tile_pool(name="wt", bufs=4) as wtp,
            tc.tile_pool(name="pr", bufs=4) as prp,
            tc.tile_pool(name="small", bufs=4) as smp,
            tc.tile_pool(name="rb", bufs=2) as rbp,
            tc.tile_pool(name="ct", bufs=3) as ctp,
            tc.tile_pool(name="psS", bufs=2, space="PSUM") as psS,
            tc.tile_pool(name="psO", bufs=2, space="PSUM") as psO,
            tc.tile_pool(name="psSum", bufs=2, space="PSUM") as psSum,
        ):
            xg_sb = res2.tile([P, NTK, GF], BF16)  # (x*c)[:, gsl] by tk chunk
            nc.sync.dma_start(out=xg_sb, in_=xgc)
            oT_sb = res2.tile([P, GH, T], BF16)    # attention out, transposed
            wo_sb = res2.tile([P, GH, D], BF16)    # Wo[:, gsl].T chunked
            nc.sync.dma_start(out=wo_sb, in_=woT)

            pending = None
            for j in range(NTQ):
                qsl = slice(j * TQ, (j + 1) * TQ)
                nkk = (j + 1) * (TQ // P)  # causal: tk chunks needed
                npair = nkk // 2
                for h in range(GH):
                    ps_o = psO.tile([P, TQ], F32, name="ps_o")
                    ps_sum = psSum.tile([1, TQ], F32, name="ps_sum")
                    # normalization of the previous head is emitted first so
                    # its DVE/gpsimd chain runs during this head's pair loop
                    # and frees the previous ps_o before our o-matmuls land
                    if pending is not None:
                        _emit_normalize(nc, smp, rbp, oT_sb, *pending)
                        pending = None
                    wps = []
                    deferred = []  # PE work lagged by one s-pair
                    for ip in range(npair):
                        ps2 = psS.tile([P, 2, TQ], F32, name="ps_s",
                                       tag="ps_s")
                        for jj in (0, 1):
                            i = 2 * ip + jj
                            g = i - (TQ // P) * j
                            lo = g * P if g > 0 else 0
                            nc.tensor.matmul(
                                ps2[:, jj, lo:],
                                kT_sb[:, h, i * P : (i + 1) * P],
                                qT_sb[:, h, j * TQ + lo : (j + 1) * TQ],
                                start=True,
                                stop=True,
                            )
                        wt2 = wtp.tile([P, 2, TQ], BF16, name="wt")
                        nc.scalar.activation(
                            wt2, ps2, mybir.ActivationFunctionType.Exp,
                            scale=sc_exp,
                        )
                        for jj in (0, 1):
                            i = 2 * ip + jj
                            g = i - (TQ // P) * j
                            if g >= 0:  # zero cols left of + on the diagonal
                                nc.gpsimd.affine_select(
                                    out=wt2[:, jj, 0 : (g + 1) * P],
                                    in_=wt2[:, jj, 0 : (g + 1) * P],
                                    pattern=[[1, (g + 1) * P]],
                                    compare_op=mybir.AluOpType.is_ge,
                                    fill=0.0,
                                    base=-(P * g),
                                    channel_multiplier=-1,
                                )
                        # emit PE work lagged one pair so the PE never
                        # waits on the exp/select chain
                        for fn in deferred:
                            fn()
                        deferred = []
                        # colsum: DVE pair+quad sums, 1 ones-matmul per quad
                        wp = prp.tile([P, TQ], BF16, name="wp")
                        nc.vector.tensor_add(wp, wt2[:, 0, :], wt2[:, 1, :])
                        wps.append(wp)

                        def _mk_o(ipc=ip, wt2c=wt2):
                            def emit():
                                for jj in (0, 1):
                                    i = 2 * ipc + jj
                                    g = i - (TQ // P) * j
                                    lo = g * P if g > 0 else 0
                                    nc.tensor.matmul(
                                        ps_o[:, lo:],
                                        xg_sb[:, i, h * HD : (h + 1) * HD],
                                        wt2c[:, jj, lo:],
                                        start=(i == 0),
                                        stop=(i == nkk - 1),
                                    )
                            return emit

                        deferred.append(_mk_o())
                        if ip % 2 == 1:
                            wq4 = prp.tile([P, TQ], BF16, name="wq4")
                            nc.vector.tensor_add(wq4, wps[-2], wps[-1])

                            def _mk_cs(iq=ip // 2, wq4c=wq4):
                                def emit():
                                    nc.tensor.matmul(
                                        ps_sum, ones_sb[:, 0:1], wq4c,
                                        start=(iq == 0),
                                        stop=(iq == npair // 2 - 1),
                                    )
                                return emit

                            deferred.append(_mk_cs())
                    for fn in deferred:
                        fn()
                    pending = (h, j, ps_o, ps_sum)
                    # interleave 2 of the 8 out-proj psum groups of the
                    # previous query chunk behind each head
                    if j > 0:
                        _emit_outproj(nc, psS, ctp, wo_sb, oT_sb, out,
                                      j - 1, (2 * h, 2 * h + 2))
                # drain the last head's normalize before C(j) can be needed
                _emit_normalize(nc, smp, rbp, oT_sb, *pending)
                pending = None
            _emit_outproj(nc, psS, ctp, wo_sb, oT_sb, out, NTQ - 1,
                          (0, DK // 2))


def _emit_normalize(nc, smp, rbp, oT_sb, h, j, ps_o, ps_sum):
    """1/colsum on one partition, gpsimd partition broadcast, then
    ps_o * recip in one DVE pass (c is folded into xgc on the host)."""
    qsl = slice(j * TQ, (j + 1) * TQ)
    recip = smp.tile([1, TQ], F32, name="recip")
    nc.vector.reciprocal_approx_fast(out=recip, in_=ps_sum)
    rb = rbp.tile([P, TQ], F32, name="rb")
    nc.gpsimd.partition_broadcast(rb, recip)
    nc.vector.tensor_mul(oT_sb[:, h, qsl], ps_o, rb)


def _emit_outproj(nc, psS, ctp, wo_sb, oT_sb, out, j, dkp_range):
    qsl = slice(j * TQ, (j + 1) * TQ)
    for dkp in range(*dkp_range):
        ps = psS.tile([P, 2, TQ], F32, name="psC_t", tag="ps_s")
        for jj in (0, 1):
            dk = 2 * dkp + jj
            for m in range(GH):
                nc.tensor.matmul(
                    ps[:, jj, :],
                    wo_sb[:, m, dk * P : (dk + 1) * P],
                    oT_sb[:, m, qsl],
                    start=(m == 0),
                    stop=(m == GH - 1),
                )
        ct = ctp.tile([P, 2, TQ], F32, name="ct")
        nc.vector.tensor_copy(out=ct, in_=ps)
        nc.sync.dma_start(out=out[:, 2 * dkp : 2 * dkp + 2, qsl], in_=ct)


def build_program():
    nc = bacc.Bacc(
        "TRN2", target_bir_lowering=False, debug=False, num_devices=NCORES
    )
    xT = nc.dram_tensor("xT", [NTQ, P, DK, TQ], BF16, kind="ExternalInput").ap()
    xgc = nc.dram_tensor("xgc", [P, NTK, GF], BF16, kind="ExternalInput").ap()
    wqT = nc.dram_tensor("wqT", [P, DK, GF], BF16, kind="ExternalInput").ap()
    wkT = nc.dram_tensor("wkT", [P, DK, GF], BF16, kind="ExternalInput").ap()
    woT = nc.dram_tensor("woT", [P, GH, D], BF16, kind="ExternalInput").ap()
    bqT = nc.dram_tensor("bqT", [HD, GH], F32, kind="ExternalInput").ap()
    bkT = nc.dram_tensor("bkT", [HD, GH], F32, kind="ExternalInput").ap()
    ones = nc.dram_tensor("ones", [P, P], BF16, kind="ExternalInput").ap()
    out = nc.dram_tensor("out", [P, DK, T], F32, kind="ExternalOutput").ap()

    with tile.TileContext(nc) as tc:
        _body(tc, xT, xgc, wqT, wkT, woT, bqT, bkT, ones, out)
    nc.compile()
    return nc


_NC_CACHE = None
LAST_RESULT = None
TRACE = False


def kernel(x, Wq, bq, Wk, bk, Wvq, bvq, v_keys, v_embed, Wo, bo):
    global _NC_CACHE, LAST_RESULT
    x = np.asarray(x, np.float32)
    Wq = np.asarray(Wq, np.float32)
    bq = np.asarray(bq, np.float32)
    Wk = np.asarray(Wk, np.float32)
    bk = np.asarray(bk, np.float32)
    v_embed = np.asarray(v_embed, np.float32)
    Wo = np.asarray(Wo, np.float32)
    bo = np.asarray(bo, np.float32)

    bf = ml_dtypes.bfloat16
    c = 2.0 * v_embed[:G].sum(axis=0)

    in_maps = []
    for core in range(NCORES):
        b, g = divmod(core, G)
        gsl = slice(g * GF, (g + 1) * GF)
        # x[b].T chunked [tci][p][dk][t]: elem = x[b][tci*TQ+t, dk*P+p]
        xT = np.ascontiguousarray(
            x[b].reshape(NTQ, TQ, DK, P).transpose(0, 3, 2, 1)
        ).astype(bf)
        # (x*c) slice chunked [p][i][f]: elem = (x*c)[b][i*P+p, gsl.start+f]
        xgc = np.ascontiguousarray(
            (x[b][:, gsl] * c[gsl]).reshape(NTK, P, GF).transpose(1, 0, 2)
        ).astype(bf)
        # Wq[gsl].T chunked [p][dk][f]: elem = Wq[gsl][f, dk*P+p]
        wqT = np.ascontiguousarray(
            Wq[gsl, :].T.reshape(DK, P, GF).transpose(1, 0, 2)
        ).astype(bf)
        wkT = np.ascontiguousarray(
            Wk[gsl, :].T.reshape(DK, P, GF).transpose(1, 0, 2)
        ).astype(bf)
        # Wo[:, gsl].T chunked [p][m][d]: elem = Wo[d, gsl.start + m*P+p]
        woT = np.ascontiguousarray(
            Wo[:, gsl].T.reshape(GH, P, D).transpose(1, 0, 2)
        ).astype(bf)
        in_maps.append(
            {
                "xT": xT,
                "xgc": xgc,
                "wqT": wqT,
                "wkT": wkT,
                "woT": woT,
                "bqT": np.ascontiguousarray(bq[gsl].reshape(GH, HD).T),
                "bkT": np.ascontiguousarray(bk[gsl].reshape(GH, HD).T),
                "ones": np.ones((P, P), bf),
            }
        )

    if _NC_CACHE is None:
        _NC_CACHE = build_program()
    res = run_bass_kernel_spmd(
        _NC_CACHE, in_maps, list(range(NCORES)), trace=TRACE
    )
    LAST_RESULT = res

    out = np.zeros((B, T, D), np.float32)
    for core in range(NCORES):
        b = core // G
        # out dram [p][dk][t]: elem = outT[dk*P+p, t] -> out[b][t, dk*P+p]
        o = res.results[core]["out"]
        out[b] += o.transpose(2, 1, 0).reshape(T, D)
    out += bo[None, None, :]
    return out


if __name__ == "__main__":
    nc = build_program()
    print("built ok")
